# revision 23
# baseline (speedup 1.0000x reference)
"""Trainium2 Bass kernel for nn_ArrivalTime (8-core data-parallel).

Math restructure (exact): with T=24 timeslots and one user per batch row,
  q = [user_feat; time_feat] @ Wq.T + bq
  scores[h,n,t] = (q . k) * scale  decomposes into
      A[h,b,t]   = ((user_row_b @ Wq_u.T + bq) . k[h,t]) * scale     (per batch row)
      C[h,tau,t] = ((ts_tau @ Wq_t.T) . k[h,t]) * scale             (per timeslot)
  so scores for token n are row  D[b(n)*24 + hour(n)]  of the [192, 96] table
  D = A + C.  The row select is computed as a one-hot matmul
      S_tile = OH0.T @ D[0:96] + OH1.T @ D[96:192] + mask.T @ mrows
  where the last term adds -1e30 to masked (token, t) scores.  After softmax
  over t within each head, out = attn_flat[n, 96] @ Vu', where
  Vu'[h*24+t, :] = v[h,t] @ Wu_h.T + bu/4 (each head's attn rows sum to 1).

Per core: shard the B axis (8 rows -> 4096 tokens).  No collectives.
Output is written bf16 and upcast to f32 on the host.
"""

import numpy as np
import ml_dtypes
from contextlib import ExitStack

import concourse.bass as bass
import concourse.mybir as mybir
import concourse.tile as tile
from concourse import bacc
from concourse.masks import make_identity
from concourse.bass_utils import run_bass_kernel_spmd

F32 = mybir.dt.float32
BF16 = mybir.dt.bfloat16
AF = mybir.ActivationFunctionType
ALU = mybir.AluOpType

D_MODEL = 1024
N_HEADS = 4
HEAD_DIM = 256
T = 24
B, S = 64, 512
NCORES = 8
BL = B // NCORES            # 8 batch rows per core
NL = BL * S                 # 4096 tokens per core
P = 128
NT = NL // P                # 32 token tiles
HT = N_HEADS * T            # 96
GRP = 8                     # token tiles per softmax group
NG = NT // GRP
KAUG = 9                    # contraction padded to 9*128 = 1152 (1024 + bias row + zeros)
SCALE = 1.0 / np.sqrt(HEAD_DIM)
NEG_BIG = np.float32(-1e30)
N_WARM = 20                 # fp32 dummy matmuls to lift the PE HAM clock gate


def build():
    nc = bacc.Bacc("TRN2", target_bir_lowering=False, debug=False)

    w1t = nc.dram_tensor("w1t", [P, KAUG, 3072], BF16, kind="ExternalInput")
    wqut = nc.dram_tensor("wqut", [P, KAUG, 1024], BF16, kind="ExternalInput")
    wut = nc.dram_tensor("wut", [P, 8, 1024], BF16, kind="ExternalInput")
    tst = nc.dram_tensor("tst", [P, KAUG, T], BF16, kind="ExternalInput")
    urt = nc.dram_tensor("urt", [P, KAUG, BL], BF16, kind="ExternalInput")
    buq = nc.dram_tensor("buq", [1, 1024], BF16, kind="ExternalInput")
    eselt = nc.dram_tensor("eselt", [32, BL * T], F32, kind="ExternalInput")
    oht0 = nc.dram_tensor("oht0", [HT, NL], BF16, kind="ExternalInput")
    ohm1 = nc.dram_tensor("ohm1", [HT + T, NL], BF16, kind="ExternalInput")
    mrows = nc.dram_tensor("mrows", [T, HT], BF16, kind="ExternalInput")
    out = nc.dram_tensor("out", [NL, D_MODEL], BF16, kind="ExternalOutput")

    with tile.TileContext(nc) as tc, ExitStack() as ctx:
        const = ctx.enter_context(tc.tile_pool(name="const", bufs=1))
        sb = ctx.enter_context(tc.tile_pool(name="sb", bufs=2))
        obp = ctx.enter_context(tc.tile_pool(name="obp", bufs=2))

        ident = const.tile([P, P], F32)
        make_identity(nc, ident[:])
        identb = const.tile([P, P], BF16)
        make_identity(nc, identb[:])

        # resident inputs (few, large, contiguous DMAs; host is partition-major)
        w1_sb = const.tile([P, KAUG, 3072], BF16)
        for c in range(3):
            nc.sync.dma_start(w1_sb[:, 3 * c:3 * (c + 1), :], w1t[:, 3 * c:3 * (c + 1), :])
        wqu_sb = const.tile([P, KAUG, 1024], BF16)
        nc.sync.dma_start(wqu_sb[:], wqut[:])
        wut_sb = const.tile([P, 8, 1024], BF16)
        nc.sync.dma_start(wut_sb[:], wut[:])
        tst_sb = const.tile([P, KAUG, T], BF16)
        nc.sync.dma_start(tst_sb[:], tst[:])
        urt_sb = const.tile([P, KAUG, BL], BF16)
        nc.sync.dma_start(urt_sb[:], urt[:])
        buq_sb = const.tile([1, 1024], BF16)
        nc.sync.dma_start(buq_sb[:], buq[:])
        eselt_sb = const.tile([32, BL * T], F32)
        nc.sync.dma_start(eselt_sb[:], eselt[:])
        oht0_sb = const.tile([HT, NL], BF16)
        nc.sync.dma_start(oht0_sb[:], oht0[:])
        ohm1_sb = const.tile([HT + T, NL], BF16)
        nc.sync.dma_start(ohm1_sb[:], ohm1[:])
        quarter = const.tile([1, HT], BF16)
        nc.vector.memset(quarter[:], 1.0 / N_HEADS)

        with tc.tile_pool(name="psp", bufs=2, space="PSUM") as psp:
            # PE warm-up while weights stream in (HAM gate: 1.2 -> 2.4 GHz).
            warm_sb = const.tile([P, 512], F32)
            nc.vector.memset(warm_sb[:], 0.0)
            warm_ps = psp.tile([P, 512], F32, tag="warm")
            for i in range(N_WARM):
                nc.tensor.matmul(warm_ps[:], ident[:], warm_sb[:],
                                 start=(i == 0), stop=(i == N_WARM - 1))
            warm_out = const.tile([P, 8], F32)
            nc.vector.tensor_copy(warm_out[:], warm_ps[:, 0:8])

            # ---- P_all = [Pq | k | v] : [24, 3072] f32 (aug row adds biases)
            # and uq = user_rows @ Wq_u.T + bq : [8, 1024] f32.
            # Column-tiled: 4 independent 24-row accumulations share one PSUM
            # pass via tile_position, quadrupling PE array occupancy.
            p_sb = const.tile([T, 3072], F32)
            uq_sb = const.tile([BL, 1024], F32)
            passes = [
                [("p", 0), ("p", 1), ("p", 2), ("p", 3)],
                [("p", 4), ("p", 5), ("u", 0), ("u", 1)],
            ]
            for pi, grp_spec in enumerate(passes):
                pp = psp.tile([P, 512], F32, tag="pre", name=f"pp{pi}")
                for kc in range(KAUG):
                    for j, (kind, n) in enumerate(grp_spec):
                        if kind == "p":
                            lhs, rhs, rows = (tst_sb[:, kc, :],
                                              w1_sb[:, kc, n * 512:(n + 1) * 512], T)
                        else:
                            lhs, rhs, rows = (urt_sb[:, kc, :],
                                              wqu_sb[:, kc, n * 512:(n + 1) * 512], BL)
                        nc.tensor.matmul(pp[32 * j:32 * j + rows, :], lhs, rhs,
                                         start=(kc == 0), stop=(kc == KAUG - 1),
                                         tile_position=(0, 32 * j),
                                         skip_group_check=True)
                for j, (kind, n) in enumerate(grp_spec):
                    if kind == "p":
                        nc.vector.tensor_copy(p_sb[:, n * 512:(n + 1) * 512],
                                              pp[32 * j:32 * j + T, :])
                    else:
                        nc.vector.tensor_copy(uq_sb[:, n * 512:(n + 1) * 512],
                                              pp[32 * j:32 * j + BL, :])

            # ---- transpose tables into contraction-major layout ----
            ca_lhsT = [const.tile([P, 2, 32], F32, name=f"ca_lhsT{h}") for h in range(N_HEADS)]
            kT = [const.tile([P, 2, T], F32, name=f"kT{h}") for h in range(N_HEADS)]
            vblkT = const.tile([P, 8, HT], BF16)
            nc.vector.memset(vblkT[:], 0.0)
            for h in range(N_HEADS):
                for cc in range(2):
                    col = h * HEAD_DIM + cc * P
                    tp = psp.tile([P, T], F32, tag="pre")
                    nc.tensor.transpose(tp[:], p_sb[:, col:col + P], ident[:T, :T])
                    nc.vector.tensor_copy(ca_lhsT[h][:, cc, 0:T], tp[:])
                    tpu = psp.tile([P, BL], F32, tag="pre")
                    nc.tensor.transpose(tpu[:], uq_sb[:, col:col + P], ident[:BL, :BL])
                    nc.vector.tensor_copy(ca_lhsT[h][:, cc, T:T + BL], tpu[:])
                    tpk = psp.tile([P, T], F32, tag="pre")
                    nc.tensor.transpose(tpk[:], p_sb[:, 1024 + col:1024 + col + P], ident[:T, :T])
                    nc.vector.tensor_copy(kT[h][:, cc, :], tpk[:])

            # ---- per-head [C; A] : [32, 24] -> AC [32, 96] ----
            ac_sb = const.tile([32, HT], F32)
            for h in range(N_HEADS):
                pca = psp.tile([32, T], F32, tag="pre")
                for cc in range(2):
                    nc.tensor.matmul(pca[:], ca_lhsT[h][:, cc, :], kT[h][:, cc, :],
                                     start=(cc == 0), stop=(cc == 1))
                nc.vector.tensor_copy(ac_sb[:, h * T:(h + 1) * T], pca[:])

            # ---- D halves (bf16, in SBUF): D = (Esel*scale) @ AC ----
            d0_sb = const.tile([96, HT], BF16)
            dm1_sb = const.tile([HT + T, HT], BF16)
            nc.sync.dma_start(dm1_sb[96:96 + T, :], mrows[:])
            for half in range(2):
                pd = psp.tile([96, HT], F32, tag="pre")
                nc.tensor.matmul(pd[:], eselt_sb[:, half * 96:(half + 1) * 96], ac_sb[:],
                                 start=True, stop=True)
                dst = d0_sb[:] if half == 0 else dm1_sb[0:96, :]
                nc.vector.tensor_copy(dst, pd[:])

            # ---- v transposes + Vu' [96, 1024] bf16 (with bu/4 row) ----
            for h in range(N_HEADS):
                for cc in range(2):
                    col = h * HEAD_DIM + cc * P
                    tpv = psp.tile([P, T], F32, tag="pre")
                    nc.tensor.transpose(tpv[:], p_sb[:, 2048 + col:2048 + col + P],
                                        ident[:T, :T])
                    nc.vector.tensor_copy(vblkT[:, h * 2 + cc, h * T:(h + 1) * T], tpv[:])
            vu_sb = const.tile([HT, 1024], BF16)
            for n in range(2):
                pv = psp.tile([HT, 512], F32, tag="pre")
                for kcc in range(8):
                    nc.tensor.matmul(pv[:], vblkT[:, kcc, :],
                                     wut_sb[:, kcc, n * 512:(n + 1) * 512],
                                     start=(kcc == 0), stop=False)
                nc.tensor.matmul(pv[:], quarter[:], buq_sb[:, n * 512:(n + 1) * 512],
                                 start=False, stop=True)
                nc.vector.tensor_copy(vu_sb[:, n * 512:(n + 1) * 512], pv[:])

        # token stage PSUM pools (precompute pool closed above)
        with tc.tile_pool(name="pst", bufs=2, space="PSUM") as pst:
            sge_tiles = {}

            def emit_scores(g):
                sge = sb.tile([P, GRP, HT], F32, tag="sge", name=f"sge{g}")
                for t in range(GRP):
                    a = g * GRP + t
                    sl = slice(a * P, (a + 1) * P)
                    psc = pst.tile([P, HT], F32, tag="sc", name=f"psc{g}_{t}")
                    nc.tensor.matmul(psc[:], oht0_sb[:, sl], d0_sb[:], start=True, stop=False)
                    nc.tensor.matmul(psc[:], ohm1_sb[:, sl], dm1_sb[:], start=False, stop=True)
                    nc.scalar.activation(sge[:, t, :], psc[:], AF.Exp)
                sge_tiles[g] = sge

            emit_scores(0)
            for g in range(NG):
                # software pipeline: queue next group's score matmuls on the PE
                # before this group's (softmax-gated) transposes, so the PE
                # instruction stream has no head-of-line stall.
                if g + 1 < NG:
                    emit_scores(g + 1)
                sge = sge_tiles.pop(g)
                scv = sge[:].rearrange("p g (h t) -> p g h t", h=N_HEADS)
                hs = sb.tile([P, GRP, N_HEADS], F32, tag="hs")
                nc.vector.reduce_sum(hs[:], scv, axis=mybir.AxisListType.X)
                nc.vector.reciprocal(hs[:], hs[:])
                rb = hs[:, :, :, None].broadcast_to([P, GRP, N_HEADS, T])
                att = sb.tile([P, GRP, HT], BF16, tag="att")
                attv = att[:].rearrange("p g (h t) -> p g h t", h=N_HEADS)
                nc.vector.tensor_tensor(out=attv, in0=scv, in1=rb, op=ALU.mult)
                ob = obp.tile([P, GRP, D_MODEL], BF16, tag="ob")
                for t in range(GRP):
                    tpa = pst.tile([HT, P], BF16, tag="tp")
                    nc.tensor.transpose(tpa[:], att[:, t, :], identb[:])
                    at = sb.tile([HT, P], BF16, tag="at")
                    nc.vector.tensor_copy(at[:], tpa[:])
                    po = pst.tile([P, D_MODEL], F32, tag="po")
                    nc.tensor.matmul(po[:, 0:512], at[:], vu_sb[:, 0:512],
                                     start=True, stop=True)
                    nc.tensor.matmul(po[:, 512:1024], at[:], vu_sb[:, 512:1024],
                                     start=True, stop=True)
                    nc.vector.tensor_copy(ob[:, t, 0:512], po[:, 0:512])
                    nc.scalar.copy(ob[:, t, 512:1024], po[:, 512:1024])
                nc.sync.dma_start(
                    out[g * GRP * P:(g + 1) * GRP * P, :].rearrange("(a p) d -> p a d", p=P),
                    ob[:])

    nc.finalize()
    return nc


def _bf16(x):
    return np.ascontiguousarray(x.astype(ml_dtypes.bfloat16))


def _pmajor(x):
    """[KC, 128, X] -> [128, KC, X] partition-major host layout."""
    return np.ascontiguousarray(np.transpose(x, (1, 0, 2)))


def prep_in_maps(inputs):
    ts = np.asarray(inputs["timeslot_embedded"], np.float32)
    user_x1 = np.asarray(inputs["user_x1"]).astype(np.int64)
    hour = np.asarray(inputs["hour_x1"]).astype(np.int64)
    mask = np.asarray(inputs["hour_mask1"]).astype(np.int64)
    up = np.asarray(inputs["up_table"], np.float32)
    Wq = np.asarray(inputs["Wq"], np.float32)
    bq = np.asarray(inputs["bq"], np.float32)
    Wk = np.asarray(inputs["Wk"], np.float32)
    bk = np.asarray(inputs["bk"], np.float32)
    Wv = np.asarray(inputs["Wv"], np.float32)
    bv = np.asarray(inputs["bv"], np.float32)
    Wu = np.asarray(inputs["Wu"], np.float32)
    bu = np.asarray(inputs["bu"], np.float32)

    Wqf = Wq.reshape(N_HEADS * HEAD_DIM, 2 * D_MODEL)
    Wq_u, Wq_t = Wqf[:, :D_MODEL], Wqf[:, D_MODEL:]
    Wkf = Wk.reshape(N_HEADS * HEAD_DIM, D_MODEL)
    Wvf = Wv.reshape(N_HEADS * HEAD_DIM, D_MODEL)

    pad = KAUG * P - D_MODEL - 1  # zero rows below the bias row
    w1 = np.concatenate([Wq_t.T, Wkf.T, Wvf.T], axis=1)                      # [1024, 3072]
    aug = np.concatenate([np.zeros(D_MODEL, np.float32), bk.ravel(), bv.ravel()])[None, :]
    w1t = _pmajor(_bf16(np.concatenate([w1, aug, np.zeros((pad, 3072), np.float32)], 0)
                        .reshape(KAUG, P, 3072)))
    wqut = _pmajor(_bf16(np.concatenate([Wq_u.T, bq.ravel()[None, :],
                                         np.zeros((pad, D_MODEL), np.float32)], 0)
                         .reshape(KAUG, P, D_MODEL)))
    wut = _pmajor(_bf16(Wu.T.reshape(8, P, D_MODEL)))
    tst = _pmajor(_bf16(np.concatenate([ts.T, np.ones((1, T), np.float32),
                                        np.zeros((pad, T), np.float32)], 0)
                        .reshape(KAUG, P, T)))
    buq = _bf16(bu[None, :])

    esel = np.zeros((BL * T, 32), np.float32)
    rows = np.arange(BL * T)
    esel[rows, rows % T] = SCALE
    esel[rows, T + rows // T] = SCALE
    eselt = np.ascontiguousarray(esel.T)

    # mrows[t', h*24+t] = -1e30 if t == t' else 0
    mrows = np.zeros((T, HT), np.float32)
    for h in range(N_HEADS):
        mrows[np.arange(T), h * T + np.arange(T)] = NEG_BIG
    mrows = _bf16(mrows)

    user_rows = up[user_x1].copy()
    user_rows[user_x1 == 0] = 0.0

    tok_b = (np.arange(NL) // S).astype(np.int64)
    in_maps = []
    for c in range(NCORES):
        ur = user_rows[c * BL:(c + 1) * BL]
        urt = _pmajor(_bf16(np.concatenate([ur.T, np.ones((1, BL), np.float32),
                                            np.zeros((pad, BL), np.float32)], 0)
                            .reshape(KAUG, P, BL)))
        hour_c = hour[c * BL:(c + 1) * BL].reshape(-1)
        idxv = tok_b * T + hour_c                      # [NL] in [0, 192)
        oh = np.zeros((2 * HT, NL), np.float32)
        oh[idxv, np.arange(NL)] = 1.0
        maskc = mask[c * BL:(c + 1) * BL].reshape(NL, T).astype(np.float32)
        ohm1c = np.concatenate([oh[HT:], maskc.T], 0)        # [120, NL]
        in_maps.append({
            "w1t": w1t, "wqut": wqut, "wut": wut, "tst": tst, "urt": urt,
            "buq": buq, "eselt": eselt, "mrows": mrows,
            "oht0": _bf16(oh[:HT]), "ohm1": _bf16(ohm1c),
        })
    return in_maps


_NC_CACHE = None


def get_nc():
    global _NC_CACHE
    if _NC_CACHE is None:
        _NC_CACHE = build()
    return _NC_CACHE


def run(inputs, trace=False, **kwargs):
    nc = get_nc()
    in_maps = prep_in_maps(inputs)
    res = run_bass_kernel_spmd(nc, in_maps, core_ids=list(range(NCORES)),
                               trace=trace, **kwargs)
    outs = [np.asarray(r["out"]) for r in res.results]
    full = np.concatenate(outs, 0).reshape(B, S, D_MODEL).astype(np.float32)
    return full, res


def kernel(**inputs):
    full, _ = run(inputs, trace=False)
    return full


# revision 25
# speedup vs baseline: 1.0292x; 1.0292x over previous
"""Trainium2 Bass kernel for nn_ArrivalTime (8-core data-parallel).

Math restructure (exact): with T=24 timeslots and one user per batch row,
  q = [user_feat; time_feat] @ Wq.T + bq
  scores[h,n,t] = (q . k) * scale  decomposes into
      A[h,b,t]   = ((user_row_b @ Wq_u.T + bq) . k[h,t]) * scale     (per batch row)
      C[h,tau,t] = ((ts_tau @ Wq_t.T) . k[h,t]) * scale             (per timeslot)
  so scores for token n are row  D[b(n)*24 + hour(n)]  of the [192, 96] table
  D = A + C.  The row select is computed as a one-hot matmul
      S_tile = OH0.T @ D[0:96] + OH1.T @ D[96:192] + mask.T @ mrows
  where the last term adds -1e30 to masked (token, t) scores.  After softmax
  over t within each head, out = attn_flat[n, 96] @ Vu', where
  Vu'[h*24+t, :] = v[h,t] @ Wu_h.T + bu/4 (each head's attn rows sum to 1).

Per core: shard the B axis (8 rows -> 4096 tokens).  No collectives.
Output is written bf16 and upcast to f32 on the host.
"""

import numpy as np
import ml_dtypes
from contextlib import ExitStack

import concourse.bass as bass
import concourse.mybir as mybir
import concourse.tile as tile
from concourse import bacc
from concourse.masks import make_identity
from concourse.bass_utils import run_bass_kernel_spmd

F32 = mybir.dt.float32
BF16 = mybir.dt.bfloat16
AF = mybir.ActivationFunctionType
ALU = mybir.AluOpType

D_MODEL = 1024
N_HEADS = 4
HEAD_DIM = 256
T = 24
B, S = 64, 512
NCORES = 8
BL = B // NCORES            # 8 batch rows per core
NL = BL * S                 # 4096 tokens per core
P = 128
NT = NL // P                # 32 token tiles
HT = N_HEADS * T            # 96
GRP = 8                     # token tiles per softmax group
NG = NT // GRP
KAUG = 9                    # contraction padded to 9*128 = 1152 (1024 + bias row + zeros)
SCALE = 1.0 / np.sqrt(HEAD_DIM)
NEG_BIG = np.float32(-1e30)
N_WARM = 20                 # fp32 dummy matmuls to lift the PE HAM clock gate


def build():
    nc = bacc.Bacc("TRN2", target_bir_lowering=False, debug=False)

    w1t = nc.dram_tensor("w1t", [P, KAUG, 3072], BF16, kind="ExternalInput")
    wqut = nc.dram_tensor("wqut", [P, KAUG, 1024], BF16, kind="ExternalInput")
    wut = nc.dram_tensor("wut", [P, 8, 1024], BF16, kind="ExternalInput")
    tst = nc.dram_tensor("tst", [P, KAUG, T], BF16, kind="ExternalInput")
    urt = nc.dram_tensor("urt", [P, KAUG, BL], BF16, kind="ExternalInput")
    buq = nc.dram_tensor("buq", [1, 1024], BF16, kind="ExternalInput")
    ohm = nc.dram_tensor("ohm", [32 + T, NL], BF16, kind="ExternalInput")
    mrows = nc.dram_tensor("mrows", [T, HT], BF16, kind="ExternalInput")
    out = nc.dram_tensor("out", [NL, D_MODEL], BF16, kind="ExternalOutput")

    with tile.TileContext(nc) as tc, ExitStack() as ctx:
        const = ctx.enter_context(tc.tile_pool(name="const", bufs=1))
        sb = ctx.enter_context(tc.tile_pool(name="sb", bufs=2))
        obp = ctx.enter_context(tc.tile_pool(name="obp", bufs=2))

        ident = const.tile([P, P], F32)
        make_identity(nc, ident[:])
        identb = const.tile([P, P], BF16)
        make_identity(nc, identb[:])

        # resident inputs (few, large, contiguous DMAs; host is partition-major)
        w1_sb = const.tile([P, KAUG, 3072], BF16)
        for c in range(3):
            nc.sync.dma_start(w1_sb[:, 3 * c:3 * (c + 1), :], w1t[:, 3 * c:3 * (c + 1), :])
        wqu_sb = const.tile([P, KAUG, 1024], BF16)
        nc.sync.dma_start(wqu_sb[:], wqut[:])
        wut_sb = const.tile([P, 8, 1024], BF16)
        nc.sync.dma_start(wut_sb[:], wut[:])
        tst_sb = const.tile([P, KAUG, T], BF16)
        nc.sync.dma_start(tst_sb[:], tst[:])
        urt_sb = const.tile([P, KAUG, BL], BF16)
        nc.sync.dma_start(urt_sb[:], urt[:])
        buq_sb = const.tile([1, 1024], BF16)
        nc.sync.dma_start(buq_sb[:], buq[:])
        ohm_sb = const.tile([32 + T, NL], BF16)
        nc.sync.dma_start(ohm_sb[:], ohm[:])
        quarter = const.tile([1, HT], BF16)
        nc.vector.memset(quarter[:], 1.0 / N_HEADS)

        with tc.tile_pool(name="psp", bufs=2, space="PSUM") as psp:
            # PE warm-up while weights stream in (HAM gate: 1.2 -> 2.4 GHz).
            warm_sb = const.tile([P, 512], F32)
            nc.vector.memset(warm_sb[:], 0.0)
            warm_ps = psp.tile([P, 512], F32, tag="warm")
            for i in range(N_WARM):
                nc.tensor.matmul(warm_ps[:], ident[:], warm_sb[:],
                                 start=(i == 0), stop=(i == N_WARM - 1))
            warm_out = const.tile([P, 8], F32)
            nc.vector.tensor_copy(warm_out[:], warm_ps[:, 0:8])

            # ---- P_all = [Pq | k | v] : [24, 3072] f32 (aug row adds biases)
            # and uq = user_rows @ Wq_u.T + bq : [8, 1024] f32.
            # Column-tiled: 4 independent 24-row accumulations share one PSUM
            # pass via tile_position, quadrupling PE array occupancy.
            p_sb = const.tile([T, 3072], F32)
            uq_sb = const.tile([BL, 1024], F32)
            passes = [
                [("p", 0), ("p", 1), ("p", 2), ("p", 3)],
                [("p", 4), ("p", 5), ("u", 0), ("u", 1)],
            ]
            for pi, grp_spec in enumerate(passes):
                pp = psp.tile([P, 512], F32, tag="pre", name=f"pp{pi}")
                for kc in range(KAUG):
                    for j, (kind, n) in enumerate(grp_spec):
                        if kind == "p":
                            lhs, rhs, rows = (tst_sb[:, kc, :],
                                              w1_sb[:, kc, n * 512:(n + 1) * 512], T)
                        else:
                            lhs, rhs, rows = (urt_sb[:, kc, :],
                                              wqu_sb[:, kc, n * 512:(n + 1) * 512], BL)
                        nc.tensor.matmul(pp[32 * j:32 * j + rows, :], lhs, rhs,
                                         start=(kc == 0), stop=(kc == KAUG - 1),
                                         tile_position=(0, 32 * j),
                                         skip_group_check=True)
                for j, (kind, n) in enumerate(grp_spec):
                    if kind == "p":
                        nc.vector.tensor_copy(p_sb[:, n * 512:(n + 1) * 512],
                                              pp[32 * j:32 * j + T, :])
                    else:
                        nc.vector.tensor_copy(uq_sb[:, n * 512:(n + 1) * 512],
                                              pp[32 * j:32 * j + BL, :])

            # ---- transpose tables into contraction-major layout ----
            ca_lhsT = [const.tile([P, 2, 32], F32, name=f"ca_lhsT{h}") for h in range(N_HEADS)]
            kT = [const.tile([P, 2, T], F32, name=f"kT{h}") for h in range(N_HEADS)]
            vblkT = const.tile([P, 8, HT], BF16)
            nc.vector.memset(vblkT[:], 0.0)
            for h in range(N_HEADS):
                for cc in range(2):
                    col = h * HEAD_DIM + cc * P
                    tp = psp.tile([P, T], F32, tag="pre")
                    nc.tensor.transpose(tp[:], p_sb[:, col:col + P], ident[:T, :T])
                    nc.vector.tensor_copy(ca_lhsT[h][:, cc, 0:T], tp[:])
                    tpu = psp.tile([P, BL], F32, tag="pre")
                    nc.tensor.transpose(tpu[:], uq_sb[:, col:col + P], ident[:BL, :BL])
                    nc.vector.tensor_copy(ca_lhsT[h][:, cc, T:T + BL], tpu[:])
                    tpk = psp.tile([P, T], F32, tag="pre")
                    nc.tensor.transpose(tpk[:], p_sb[:, 1024 + col:1024 + col + P], ident[:T, :T])
                    nc.vector.tensor_copy(kT[h][:, cc, :], tpk[:])

            # ---- per-head [C; A] : [32, 24] -> acm rows 0:32 (bf16);
            #      mask bias rows (mrows) at partitions 32:56 ----
            acm_sb = const.tile([32 + T, HT], BF16)
            nc.sync.dma_start(acm_sb[32:32 + T, :], mrows[:])
            for h in range(N_HEADS):
                pca = psp.tile([32, T], F32, tag="pre")
                for cc in range(2):
                    nc.tensor.matmul(pca[:], ca_lhsT[h][:, cc, :], kT[h][:, cc, :],
                                     start=(cc == 0), stop=(cc == 1))
                nc.vector.tensor_copy(acm_sb[0:32, h * T:(h + 1) * T], pca[:])

            # ---- v transposes + Vu' [96, 1024] bf16 (with bu/4 row) ----
            for h in range(N_HEADS):
                for cc in range(2):
                    col = h * HEAD_DIM + cc * P
                    tpv = psp.tile([P, T], F32, tag="pre")
                    nc.tensor.transpose(tpv[:], p_sb[:, 2048 + col:2048 + col + P],
                                        ident[:T, :T])
                    nc.vector.tensor_copy(vblkT[:, h * 2 + cc, h * T:(h + 1) * T], tpv[:])
            vu_sb = const.tile([HT, 1024], BF16)
            for n in range(2):
                pv = psp.tile([HT, 512], F32, tag="pre")
                for kcc in range(8):
                    nc.tensor.matmul(pv[:], vblkT[:, kcc, :],
                                     wut_sb[:, kcc, n * 512:(n + 1) * 512],
                                     start=(kcc == 0), stop=False)
                nc.tensor.matmul(pv[:], quarter[:], buq_sb[:, n * 512:(n + 1) * 512],
                                 start=False, stop=True)
                nc.vector.tensor_copy(vu_sb[:, n * 512:(n + 1) * 512], pv[:])

        # token stage PSUM pools (precompute pool closed above)
        with tc.tile_pool(name="pst", bufs=2, space="PSUM") as pst:
            sge_tiles = {}

            def emit_scores(g):
                sge = sb.tile([P, GRP, HT], F32, tag="sge", name=f"sge{g}")
                for t in range(GRP):
                    a = g * GRP + t
                    sl = slice(a * P, (a + 1) * P)
                    psc = pst.tile([P, HT], F32, tag="sc", name=f"psc{g}_{t}")
                    nc.tensor.matmul(psc[:], ohm_sb[:, sl], acm_sb[:], start=True, stop=True)
                    nc.scalar.activation(sge[:, t, :], psc[:], AF.Exp)
                sge_tiles[g] = sge

            emit_scores(0)
            for g in range(NG):
                # software pipeline: queue next group's score matmuls on the PE
                # before this group's (softmax-gated) transposes, so the PE
                # instruction stream has no head-of-line stall.
                if g + 1 < NG:
                    emit_scores(g + 1)
                sge = sge_tiles.pop(g)
                scv = sge[:].rearrange("p g (h t) -> p g h t", h=N_HEADS)
                hs = sb.tile([P, GRP, N_HEADS], F32, tag="hs")
                nc.vector.reduce_sum(hs[:], scv, axis=mybir.AxisListType.X)
                nc.vector.reciprocal(hs[:], hs[:])
                rb = hs[:, :, :, None].broadcast_to([P, GRP, N_HEADS, T])
                att = sb.tile([P, GRP, HT], BF16, tag="att")
                attv = att[:].rearrange("p g (h t) -> p g h t", h=N_HEADS)
                nc.vector.tensor_tensor(out=attv, in0=scv, in1=rb, op=ALU.mult)
                ob = obp.tile([P, GRP, D_MODEL], BF16, tag="ob")
                for t in range(GRP):
                    tpa = pst.tile([HT, P], BF16, tag="tp")
                    nc.tensor.transpose(tpa[:], att[:, t, :], identb[:])
                    at = sb.tile([HT, P], BF16, tag="at")
                    nc.vector.tensor_copy(at[:], tpa[:])
                    po = pst.tile([P, D_MODEL], F32, tag="po")
                    nc.tensor.matmul(po[:, 0:512], at[:], vu_sb[:, 0:512],
                                     start=True, stop=True)
                    nc.tensor.matmul(po[:, 512:1024], at[:], vu_sb[:, 512:1024],
                                     start=True, stop=True)
                    nc.vector.tensor_copy(ob[:, t, 0:512], po[:, 0:512])
                    nc.scalar.copy(ob[:, t, 512:1024], po[:, 512:1024])
                nc.sync.dma_start(
                    out[g * GRP * P:(g + 1) * GRP * P, :].rearrange("(a p) d -> p a d", p=P),
                    ob[:])

    nc.finalize()
    return nc


def _bf16(x):
    return np.ascontiguousarray(x.astype(ml_dtypes.bfloat16))


def _pmajor(x):
    """[KC, 128, X] -> [128, KC, X] partition-major host layout."""
    return np.ascontiguousarray(np.transpose(x, (1, 0, 2)))


def prep_in_maps(inputs):
    ts = np.asarray(inputs["timeslot_embedded"], np.float32)
    user_x1 = np.asarray(inputs["user_x1"]).astype(np.int64)
    hour = np.asarray(inputs["hour_x1"]).astype(np.int64)
    mask = np.asarray(inputs["hour_mask1"]).astype(np.int64)
    up = np.asarray(inputs["up_table"], np.float32)
    Wq = np.asarray(inputs["Wq"], np.float32)
    bq = np.asarray(inputs["bq"], np.float32)
    Wk = np.asarray(inputs["Wk"], np.float32)
    bk = np.asarray(inputs["bk"], np.float32)
    Wv = np.asarray(inputs["Wv"], np.float32)
    bv = np.asarray(inputs["bv"], np.float32)
    Wu = np.asarray(inputs["Wu"], np.float32)
    bu = np.asarray(inputs["bu"], np.float32)

    Wqf = Wq.reshape(N_HEADS * HEAD_DIM, 2 * D_MODEL)
    Wq_u, Wq_t = Wqf[:, :D_MODEL], Wqf[:, D_MODEL:]
    Wkf = Wk.reshape(N_HEADS * HEAD_DIM, D_MODEL)
    Wvf = Wv.reshape(N_HEADS * HEAD_DIM, D_MODEL)

    pad = KAUG * P - D_MODEL - 1  # zero rows below the bias row
    w1 = np.concatenate([Wq_t.T, Wkf.T, Wvf.T], axis=1)                      # [1024, 3072]
    aug = np.concatenate([np.zeros(D_MODEL, np.float32), bk.ravel(), bv.ravel()])[None, :]
    w1t = _pmajor(_bf16(np.concatenate([w1, aug, np.zeros((pad, 3072), np.float32)], 0)
                        .reshape(KAUG, P, 3072)))
    wqut = _pmajor(_bf16(np.concatenate([Wq_u.T, bq.ravel()[None, :],
                                         np.zeros((pad, D_MODEL), np.float32)], 0)
                         .reshape(KAUG, P, D_MODEL)))
    wut = _pmajor(_bf16(Wu.T.reshape(8, P, D_MODEL)))
    tst = _pmajor(_bf16(np.concatenate([ts.T, np.ones((1, T), np.float32),
                                        np.zeros((pad, T), np.float32)], 0)
                        .reshape(KAUG, P, T)))
    buq = _bf16(bu[None, :])

    # mrows[t', h*24+t] = -1e30 if t == t' else 0
    mrows = np.zeros((T, HT), np.float32)
    for h in range(N_HEADS):
        mrows[np.arange(T), h * T + np.arange(T)] = NEG_BIG
    mrows = _bf16(mrows)

    user_rows = up[user_x1].copy()
    user_rows[user_x1 == 0] = 0.0

    tok_b = (np.arange(NL) // S).astype(np.int64)
    in_maps = []
    for c in range(NCORES):
        ur = user_rows[c * BL:(c + 1) * BL]
        urt = _pmajor(_bf16(np.concatenate([ur.T, np.ones((1, BL), np.float32),
                                            np.zeros((pad, BL), np.float32)], 0)
                            .reshape(KAUG, P, BL)))
        hour_c = hour[c * BL:(c + 1) * BL].reshape(-1)
        ohe = np.zeros((32 + T, NL), np.float32)
        ohe[hour_c, np.arange(NL)] = SCALE             # tau selector -> C rows
        ohe[T + tok_b, np.arange(NL)] = SCALE          # b selector -> A rows
        maskc = mask[c * BL:(c + 1) * BL].reshape(NL, T).astype(np.float32)
        ohe[32:32 + T, :] = maskc.T                    # mask rows -> mrows
        in_maps.append({
            "w1t": w1t, "wqut": wqut, "wut": wut, "tst": tst, "urt": urt,
            "buq": buq, "mrows": mrows, "ohm": _bf16(ohe),
        })
    return in_maps


_NC_CACHE = None


def get_nc():
    global _NC_CACHE
    if _NC_CACHE is None:
        _NC_CACHE = build()
    return _NC_CACHE


def run(inputs, trace=False, **kwargs):
    nc = get_nc()
    in_maps = prep_in_maps(inputs)
    res = run_bass_kernel_spmd(nc, in_maps, core_ids=list(range(NCORES)),
                               trace=trace, **kwargs)
    outs = [np.asarray(r["out"]) for r in res.results]
    full = np.concatenate(outs, 0).reshape(B, S, D_MODEL).astype(np.float32)
    return full, res


def kernel(**inputs):
    full, _ = run(inputs, trace=False)
    return full
